# revision 57
# baseline (speedup 1.0000x reference)
"""IntersectionLoss Trainium2 kernel.

Math: loss_n = maskedmean_j relu(R + S*log(sum_i exp(-|t2_nj - t1_ni|^2/S) * m1_i + eps))
Key identity: |t2_j - t1_i|^2 = n2_j + n1_i - 2*t2_j.t1_i, so the full exponent
  x_ji = 2*t2_j.t1_i/S - n1_i/S + ln m1_i - n2_j/S
is a K=5 bilinear form: with augmented rows (gamma^2 = 2/(S*128))
  t1aug = [gamma*x, gamma*y, gamma*z, (-n1_i/S + ln m1_i)/128, 1]
  t2aug = [gamma*x, gamma*y, gamma*z, 1,            1 - n2_j/(S*128)]
one PE matmul produces t_ji = 1 + x_ji/128 directly in PSUM.

The L1-reduction of exp(x) then runs on TWO engines in parallel (each plane's
two 1024-wide PSUM chunks go to different engines):
  - ACT chunks: scalar.activation Exp with scale=128, bias=-128 (exp(128t-128)
    == exp(x)) and the sum riding accum_out — 1 elem/lane/cycle at 1.2 GHz.
  - GPSIMD chunks: tensor_scalar pow(t, 128)*1 with accum_out computes
    t^128 = (1+x/128)^128 ~ exp(x) plus the row sum in one ucode instruction
    (vpowf on the 8 Q7 cores).
The (1+x/128)^128 approximation under-counts acc by <~1% in the mass-carrying
terms; the resulting loss error is ~1e-3 relative (tolerance 2e-2).

Sharding: data-parallel over N=16 across 8 cores (2 batches per core). Final
log/relu/masked-mean over the (N,2048) accumulator runs on host in float64.
"""

import sys

sys.path.insert(0, "/opt/trn_rl_repo")

import numpy as np

import concourse.bass as bass
import concourse.tile as tile
from concourse import mybir
from concourse.bass_utils import run_bass_kernel_spmd

RADIUS = 1.0
SIGMA = 2.5
EPSILON = 1e-12

N, L1, L2 = 16, 2048, 2048
NCORES = 8
NB = N // NCORES  # batches per core
P = 128
A = L2 // P  # 16 j-tiles per batch
F32 = mybir.dt.float32
BF16 = mybir.dt.bfloat16
F32R = mybir.dt.float32r
AF = mybir.ActivationFunctionType

_CACHE = {}

H = 2  # chunks per plane: ACT eats half 0, GPSIMD half 1
CH = L1 // H  # 1024 elems = 2 PSUM banks; 4 rotating buffers = 8 banks
# ACT sustains ~1184ns/chunk, the DVE-copy+GPSIMD-pow lane ~1550ns, so ACT
# additionally takes the GPSIMD half of 6 planes (38/26 split); planes 30/31
# are stolen so the slower pow lane is not the last to finish (sim-swept).
ACT_STEAL = frozenset({5, 11, 17, 23, 30, 31})


def _build_program():
    nc = bass.Bass()
    # taug[b, k, s, i]: s=0 -> t1aug row k, s=1 -> t2aug row k (i in 0..2047)
    taug_d = nc.declare_dram_parameter("taug", (NB, 5, 2, L1), F32R, isOutput=False)
    acc_a_d = nc.declare_dram_parameter("acc_a", (P, NB * A * H), F32, isOutput=True)
    acc_v_d = nc.declare_dram_parameter("acc_v", (P, NB * A * H), F32, isOutput=True)
    NMM = CH // 512  # matmuls per chunk

    with tile.TileContext(nc) as tc:
        with (
            tc.tile_pool(name="consts", bufs=1) as consts,
            tc.tile_pool(name="sb", bufs=2) as sb,
            # one staging tile per GPSIMD chunk (never reused): the DVE copy
            # then waits only on its PE fill — a rotating pool would add a
            # WAR wait and overflow the 1-wait queue structs
            tc.tile_pool(name="stage", bufs=NB * A - len(ACT_STEAL)) as stage,
            tc.tile_pool(name="ps", bufs=4, space="PSUM") as ps,
        ):
            # broadcast exponent operand for the GPSIMD pow tensor_tensor
            c128 = consts.tile([P, 1], F32)
            nc.gpsimd.memset(c128[:], 128.0)
            # ACT bias const (exp(128t - 128)); memset is tile-tracked so the
            # first activation gets a proper Pool->ACT dependency
            bias_t = consts.tile([P, 1], F32)
            nc.gpsimd.memset(bias_t[:], -128.0)

            # single input DMA: one completion semaphore, so every matmul
            # carries at most one sync wait (the PE Matmult queue struct
            # fits only one; see _elide_redundant_matmul_waits)
            tT = consts.tile([5, NB * 2 * L1], F32R)
            nc.sync.dma_start(
                out=tT.rearrange("k (b s i) -> k b s i", b=NB, s=2),
                in_=taug_d.rearrange("b k s i -> k b s i"),
            )

            # PE warm-up: tiny matmuls start the pstate ramp (full clock needs
            # ~3us of continuous busy) before the real fills arrive. The
            # operand is an UNTRACKED SBUF alloc read uninitialized — no
            # dependency, so the ramp starts right after init; the outputs
            # are garbage in ring slots that real fills overwrite.
            warm_ap = nc.alloc_sbuf_tensor("warm_fodder", [5, 192], F32R).ap()
            for w in range(8):
                gw = ps.tile([P, CH], F32, tag="ps")
                nc.tensor.matmul(
                    gw[:, :64], warm_ap[:, :128], warm_ap[:, 128:192],
                    start=True, stop=True,
                )

            # separate per-engine accumulators so ACT and DVE never touch the
            # same tile; one column per chunk, merged on host. Zeroed on
            # device: each engine writes only its own columns and the host
            # sums both tensors — the DMA'd-out bytes of the other engine's
            # columns must be 0.0, not uninitialized SBUF.
            acc_act = sb.tile([P, NB * A * H], F32, tag="acc_act")
            acc_dve = sb.tile([P, NB * A * H], F32, tag="acc_dve")
            # acc_act memset LAST: the table-warm exp below waits on it, and
            # that single Pool-sem wait then also covers the acc_dve memset
            nc.gpsimd.memset(acc_dve[:], 0.0)
            nc.gpsimd.memset(acc_act[:], 0.0)
            # warm the Exp table while the input DMA is in flight (real HW
            # charges ~1.3us for the first table load); its accum lands in
            # acc_act[:, 0] — overwritten by the real chunk — which also
            # absorbs the acc-memset wait onto the ACT queue, keeping the
            # first real Activation at one sync wait
            tiny = consts.tile([P, 1], F32)
            nc.scalar.activation(
                tiny[:],
                bias_t[:],
                AF.Exp,
                bias=bias_t[:],
                scale=0.0,
                accum_out=acc_act[:, 0:1],
            )
            pending_reduce = []  # deferred (dump, col) so DVE copies never
            # queue behind a reduce that is still waiting on the GPSIMD pow

            def flush_reduce():
                for dump, col in pending_reduce:
                    nc.vector.tensor_scalar(
                        dump[:],
                        dump[:],
                        1.0,
                        0.0,
                        mybir.AluOpType.mult,
                        mybir.AluOpType.add,
                        accum_out=acc_dve[:, col : col + 1],
                    )
                pending_reduce.clear()

            for b in range(NB):
                for jt in range(A):
                    lhsT = tT[:, (2 * b + 1) * L1 + jt * P : (2 * b + 1) * L1 + (jt + 1) * P]
                    plane = b * A + jt
                    for h in range(H):
                        g = ps.tile([P, CH], F32, tag="ps")
                        for it in range(NMM):
                            i0 = 2 * b * L1 + h * CH + it * 512
                            nc.tensor.matmul(
                                g[:, it * 512 : (it + 1) * 512],
                                lhsT,
                                tT[:, i0 : i0 + 512],
                                start=True,
                                stop=True,
                            )
                        col = plane * H + h
                        if h == 0 or plane in ACT_STEAL:
                            nc.scalar.activation(
                                g[:],
                                g[:],
                                AF.Exp,
                                bias=bias_t[:],
                                scale=128.0,
                                accum_out=acc_act[:, col : col + 1],
                            )
                        else:
                            # GPSIMD cannot read PSUM: DVE stages the chunk to
                            # SBUF (f32 — pow amplifies rounding x128, bf16
                            # staging would cost ~25% accuracy), then GPSIMD
                            # computes pow(t,128)*1 with the row-sum riding
                            # accum_out. Tiles rotate (3 bufs) so the copy,
                            # the pow, and the next copy pipeline.
                            scr = stage.tile([P, CH], F32, tag="scr")
                            nc.vector.tensor_scalar(
                                scr[:], g[:], 1.0, None, mybir.AluOpType.mult
                            )
                            dump = stage.tile([P, CH], BF16, tag="dump")
                            nc.gpsimd.tensor_tensor(
                                dump[:],
                                scr[:],
                                c128[:].to_broadcast((P, CH)),
                                mybir.AluOpType.pow,
                            )
                            flush_reduce()
                            pending_reduce.append((dump, col))
                    if plane == NB * A // 2 - 1:
                        # drain the first batch's accumulators while the second
                        # batch computes; only the tail columns ride the final DMA
                        flush_reduce()
                        half = A * H
                        nc.sync.dma_start(
                            out=acc_a_d[:, :half], in_=acc_act[:, :half]
                        )
                        nc.sync.dma_start(
                            out=acc_v_d[:, :half], in_=acc_dve[:, :half]
                        )
            flush_reduce()
            half = A * H
            # acc_v first: the pow lane finishes ~0.7us before ACT, so its
            # descriptor processes on the serial HWDGE queue while the last
            # ACT chunks are still running
            nc.sync.dma_start(out=acc_v_d[:, half:], in_=acc_dve[:, half:])
            nc.sync.dma_start(out=acc_a_d[:, half:], in_=acc_act[:, half:])

    _elide_redundant_matmul_waits(nc)
    return nc


def _elide_redundant_matmul_waits(nc):
    """Drop semaphore waits on Matmult instrs that are transitively implied by
    their other waits (Tile emits per-proc-minimal, not transitively-minimal,
    waits; the PE Matmult queue struct only fits one sync wait command).

    Soundness: a wait (S, v) is removed only if chaining (a) same-engine
    in-order start/completion and (b) the completion vector clocks of the
    producers of the REMAINING waits already guarantees S >= v.
    """

    def merge(dst, src):
        for k, v in src.items():
            if dst.get(k, 0) < v:
                dst[k] = v

    all_insts = []
    for bb in nc.bb_map.values():
        all_insts.extend(bb.bb.instructions)
    if True:
        insts = all_insts
        n = len(insts)
        # cumulative updater ticks per semaphore
        sem_updaters = {}  # sem -> list of (cum_value, idx)
        sem_cum = {}
        idx_updates = [[] for _ in range(n)]  # idx -> [(sem, cum_after)]
        for idx, inst in enumerate(insts):
            si = inst.sync_info
            if not si:
                continue
            for u in si.on_update:
                s = u.ant_name
                v = getattr(u, "update_value", None) or 1
                c = sem_cum.get(s, 0) + v
                sem_cum[s] = c
                sem_updaters.setdefault(s, []).append((c, idx))
                idx_updates[idx].append((s, c))

        def producer_of(s, v):
            for c, uidx in sem_updaters.get(s, ()):
                if c >= v:
                    return uidx
            return None

        start_clock = [dict() for _ in range(n)]
        comp_clock = [dict() for _ in range(n)]
        for _ in range(3):
            prev_start = {}
            prev_comp = {}
            for idx, inst in enumerate(insts):
                e = str(inst.engine)
                sc = dict(prev_start.get(e, {}))
                si = inst.sync_info
                if si:
                    for w in si.on_wait:
                        s, v = w.ant_name, w.wait_value
                        if sc.get(s, 0) < v:
                            sc[s] = v
                        p = producer_of(s, v)
                        if p is not None:
                            merge(sc, comp_clock[p])
                cc = dict(sc)
                merge(cc, prev_comp.get(e, {}))
                for s, c in idx_updates[idx]:
                    if cc.get(s, 0) < c:
                        cc[s] = c
                start_clock[idx] = sc
                comp_clock[idx] = cc
                prev_start[e] = sc
                prev_comp[e] = cc

        # drop same-engine waits on multi-wait instructions: each engine
        # executes its queue in order, so a wait whose updaters are all
        # earlier instructions of the same engine is redundant (most queue
        # structs only fit one sync wait)
        for idx, inst in enumerate(insts):
            si = inst.sync_info
            if not si or len(si.on_wait) <= 1:
                continue
            eng = str(inst.engine)
            kept = []
            for w in si.on_wait:
                need = [
                    uidx
                    for c, uidx in sem_updaters.get(w.ant_name, ())
                    if 1 <= c <= w.wait_value
                ]
                if need and all(
                    uidx < idx and str(insts[uidx].engine) == eng for uidx in need
                ):
                    continue  # implied by same-engine program order
                kept.append(w)
            if not kept:
                kept = [si.on_wait[-1]]
            if len(kept) < len(si.on_wait):
                si.on_wait = kept
                inst.sync_info = si

        # elide waits implied by remaining waits + engine order
        prev_start = {}
        for idx, inst in enumerate(insts):
            e = str(inst.engine)
            si = inst.sync_info
            if si and len(si.on_wait) > 1:
                waits = list(si.on_wait)
                kept = list(waits)
                for w in waits:
                    if len(kept) <= 1:
                        break
                    others = [x for x in kept if x is not w]
                    implied = dict(prev_start.get(e, {}))
                    for o in others:
                        if implied.get(o.ant_name, 0) < o.wait_value:
                            implied[o.ant_name] = o.wait_value
                        p = producer_of(o.ant_name, o.wait_value)
                        if p is not None:
                            merge(implied, comp_clock[p])
                    if implied.get(w.ant_name, 0) >= w.wait_value:
                        kept = others
                if len(kept) < len(waits):
                    si.on_wait = kept
                    inst.sync_info = si
            sc = dict(prev_start.get(e, {}))
            if si:
                for w in si.on_wait:
                    if sc.get(w.ant_name, 0) < w.wait_value:
                        sc[w.ant_name] = w.wait_value
                    p = producer_of(w.ant_name, w.wait_value)
                    if p is not None:
                        merge(sc, comp_clock[p])
            prev_start[e] = sc


def _prep(t1, t2, mask1):
    """Build taug (N,5,2,L1) on host; the matmul then yields t = 1 + x/128."""
    n1 = np.einsum("nik,nik->ni", t1, t1)  # (N, L1)
    n2 = np.einsum("njk,njk->nj", t2, t2)  # (N, L2)
    with np.errstate(divide="ignore"):
        # clamp so masked-out (m1=0) entries give |t| < 1 -> t^128 ~ 0
        lnm1 = np.maximum(np.log(mask1), -120.0)
    gamma = np.sqrt(2.0 / (SIGMA * 128.0)).astype(np.float32)
    taug = np.empty((N, 5, 2, L1), np.float32)
    taug[:, 0:3, 0, :] = t1.transpose(0, 2, 1) * gamma
    taug[:, 3, 0, :] = (-n1 / SIGMA + lnm1) / 128.0
    taug[:, 4, 0, :] = 1.0
    taug[:, 0:3, 1, :] = t2.transpose(0, 2, 1) * gamma
    taug[:, 3, 1, :] = 1.0
    taug[:, 4, 1, :] = 1.0 - n2 / (SIGMA * 128.0)
    return taug


def _make_in_maps(t1, t2, mask1, mask2):
    t1 = np.asarray(t1, dtype=np.float32)
    t2 = np.asarray(t2, dtype=np.float32)
    mask1 = np.asarray(mask1, dtype=np.float32)
    taug = _prep(t1, t2, mask1)
    return [{"taug": taug[c * NB : (c + 1) * NB]} for c in range(NCORES)]


def kernel(t1, t2, mask1, mask2):
    if "nc" not in _CACHE:
        _CACHE["nc"] = _build_program()
    nc = _CACHE["nc"]

    in_maps = _make_in_maps(t1, t2, mask1, mask2)
    res = run_bass_kernel_spmd(nc, in_maps, list(range(NCORES)))

    # per core: acc[p, (b*A+jt)*H + h], j = jt*128+p; each column written by
    # exactly one engine (the other output stays zero), so summing the two
    # outputs and then the H halves merges everything
    acc = np.stack(
        [
            (r["acc_a"] + r["acc_v"]).reshape(P, NB, A, H).sum(axis=-1)
            for r in res.results
        ]
    )  # (C,P,NB,A)
    acc_full = acc.transpose(0, 2, 3, 1).reshape(N, L2).astype(np.float64)

    d = RADIUS + SIGMA * np.log(acc_full + EPSILON)
    d = np.maximum(d, 0.0)
    m2 = np.asarray(mask2).astype(np.float64)
    loss = (d * m2).sum(axis=-1) / m2.sum(axis=-1)
    return loss.astype(np.float32)
